# revision 4
# baseline (speedup 1.0000x reference)
"""Trainium2 Bass kernel for nn_Conv4Pim_group_arr_v3 (PIM-style grouped quantized conv).

Computation (see reference):
  - x [16,256,56,56] f32, weight [256,256,3,3], per-group (G=4, 64 ic each) LSQ
    quantization: weights to integer levels {0..3} (pos/neg split), partial-sum conv
    outputs rounded to int levels in [-128,127] and rescaled, accumulated over groups.

Strategy: data-parallel over batch (2 images per core, 8 cores, no collectives).
Per core, per (img, group, och-tile-of-512, sptile-of-8-rows):
  - 4 full fp16 matmuls (K=128: two conv taps x 64 ic stacked; x stored as [A | A+1]
    and [A | A+58] shifted copies so taps pair up) accumulate 8 of the 9 conv taps
    into one PSUM tile [128 och, 464=8*58 padded-row columns].
  - The 9th tap (2,2) is K=64 only; two och-tiles' tap-(2,2) matmuls are packed into
    one PE pass via 64x128 row tiling (tile_position (0,0)/(64,0)), batched per
    2-sp window to amortize the PE mode-switch drain.
  - ACT: Copy(psum * (w_scale/ps_scale)) with int8 output = round-half-even +
    saturate to [-128,127] in one op (verified on HW) == the LSQ psum quantizer.
  - DVE scalar_tensor_tensor: acc_fp16 += q_int8 * (+-ps_scale).
Output fp16 -> host f32.
"""

import numpy as np

import concourse.mybir as mybir
import concourse.tile as tile
from concourse import bacc
from concourse.bass_utils import run_bass_kernel_spmd

F32 = mybir.dt.float32
F16 = mybir.dt.float16
I8 = mybir.dt.int8

B, IC, H, W = 16, 256, 56, 56
OC = 256
G = 4
CG = 64  # ic per group
K = 3
QP_W = 3  # 2**2 - 1
N_CORES = 8
BPC = B // N_CORES  # images per core

PW = W + 2  # 58 padded width
PH = H + 2
FLAT = PW * PH  # 3364
FLATP = FLAT + 4  # padded to 3368 for tap-read overhang
SP = 7  # spatial tiles of 8 output rows
ROWS = 8
NCOL = ROWS * PW  # 464 columns per psum tile
OC4 = 4  # och tiles of 128 over 512 (pos|neg x 256)
WINDOWS = [(0, 1), (2, 3), (4, 5), (6,)]

_nc_cache = {}


def _build_nc():
    nc = bacc.Bacc(
        "TRN2",
        target_bir_lowering=False,
        debug=False,
        enable_asserts=True,
        num_devices=N_CORES,
    )

    xt1_d = nc.dram_tensor("xt1", [BPC, G, 128, FLATP], F16, kind="ExternalInput").ap()
    xt2_d = nc.dram_tensor("xt2", [BPC, G, 128, FLATP], F16, kind="ExternalInput").ap()
    # main slots: 4 per (g, oc4); tap-(2,2) packed separately for row tiling
    wts_d = nc.dram_tensor("wts", [128, G * OC4 * 4 * 128], F16, kind="ExternalInput").ap()
    w22_d = nc.dram_tensor("w22", [128, G * 2 * 128], F16, kind="ExternalInput").ap()
    scl_d = nc.dram_tensor("scl", [128, 2 * G * OC4], F32, kind="ExternalInput").ap()
    # padded output: [img, oct, sp, och, 464 cols]; host strips the pad columns
    out_d = nc.dram_tensor("out", [BPC, 2, SP, 128, NCOL], F16, kind="ExternalOutput").ap()

    W1 = 4 * 128  # one (g, oc4) main-slot weight slice
    WG = OC4 * W1  # one group of main slots

    with tile.TileContext(nc) as tc:
        with (
            tc.tile_pool(name="xp", bufs=1) as xp,
            tc.tile_pool(name="wp", bufs=1) as wp,
            tc.tile_pool(name="accp", bufs=2) as accp,
            tc.tile_pool(name="qp", bufs=8) as qp,
            tc.tile_pool(name="psum", bufs=8, space="PSUM") as pp,
        ):
            wts = wp.tile([128, G * OC4 * 4 * 128], F16, tag="wts")
            w22 = wp.tile([128, G * 2 * 128], F16, tag="w22")
            scl = wp.tile([128, 2 * G * OC4], F32, tag="scl")

            # Startup-critical DMA schedule over four queues (sync + scalar =
            # HWDGE, gpsimd q0/q1 = SWDGE). First window (img0,g0,sp0-1) needs
            # t1/t2 cols < 1046 and the g0 weight slices; chunks ordered to
            # stay ahead of consumption.
            C1, C2, C3 = 582, 1046, 2500

            xt = {}
            t1_first = xp.tile([128, FLATP], F16, tag="t1_0_0")
            t2_first = xp.tile([128, FLATP], F16, tag="t2_0_0")
            xt[0, 0] = (t1_first, t2_first)

            nc.sync.dma_start(scl[:], scl_d[:])
            nc.sync.dma_start(wts[:, :W1], wts_d[:, :W1])
            nc.sync.dma_start(t1_first[:, :C1], xt1_d[0, 0, :, :C1])
            for i in range(1, OC4):
                nc.sync.dma_start(wts[:, i * W1 : (i + 1) * W1], wts_d[:, i * W1 : (i + 1) * W1])
            nc.sync.dma_start(t1_first[:, C1:C2], xt1_d[0, 0, :, C1:C2])
            nc.sync.dma_start(t1_first[:, C2:C3], xt1_d[0, 0, :, C2:C3])
            nc.sync.dma_start(t1_first[:, C3:], xt1_d[0, 0, :, C3:])

            # scalar HWDGE: first-set t2 chunks + startup weights (issued before
            # the scalar engine's first ACT op, transfers run async)
            nc.scalar.dma_start(t2_first[:, :C1], xt2_d[0, 0, :, :C1])
            nc.scalar.dma_start(w22[:, :256], w22_d[:, :256])
            nc.scalar.dma_start(t2_first[:, C1:C2], xt2_d[0, 0, :, C1:C2])
            nc.scalar.dma_start(t2_first[:, C2:C3], xt2_d[0, 0, :, C2:C3])
            nc.scalar.dma_start(t2_first[:, C3:], xt2_d[0, 0, :, C3:])
            nc.scalar.dma_start(wts[:, WG : 2 * WG], wts_d[:, WG : 2 * WG])
            nc.scalar.dma_start(w22[:, 256:], w22_d[:, 256:])

            # gpsimd q0: remaining weights, then t2 streams
            nc.gpsimd.dma_start(wts[:, 2 * WG :], wts_d[:, 2 * WG :])

            for img in range(BPC):
                for g in range(G):
                    if (img, g) in xt:
                        continue
                    t1 = xp.tile([128, FLATP], F16, tag=f"t1_{img}_{g}")
                    t2 = xp.tile([128, FLATP], F16, tag=f"t2_{img}_{g}")
                    nc.sync.dma_start(t1[:], xt1_d[img, g])
                    nc.gpsimd.dma_start(t2[:], xt2_d[img, g])
                    xt[img, g] = (t1, t2)

            def wslice(g, oc4, s):
                i = ((g * OC4) + oc4) * 4 + s
                return wts[:, i * 128 : (i + 1) * 128]

            out_q = [nc.sync, nc.gpsimd]

            for img in range(BPC):
                acc = {}
                for oct in range(2):
                    for sp in range(SP):
                        a_t = accp.tile([128, NCOL], F16, tag=f"acc{oct}_{sp}")
                        acc[oct, sp] = a_t

                for g in range(G):
                    t1, t2 = xt[img, g]
                    for window in WINDOWS:
                        ps = {}
                        for sp in window:
                            r0 = sp * ROWS
                            for oc4 in range(OC4):
                                p = pp.tile([128, NCOL], F32, tag="ps")
                                ps[oc4, sp] = p
                                for s in range(3):
                                    nc.tensor.matmul(
                                        p[:],
                                        wslice(g, oc4, s),
                                        t1[:, (r0 + s) * PW : (r0 + s) * PW + NCOL],
                                        start=(s == 0),
                                        stop=False,
                                    )
                                nc.tensor.matmul(
                                    p[:],
                                    wslice(g, oc4, 3),
                                    t2[:, r0 * PW + 2 : r0 * PW + 2 + NCOL],
                                    start=False,
                                    stop=False,
                                )
                        # tap-(2,2) for all oc4 of the window: 64x128 row-tiled
                        # pairs (T0 reads partitions 0-63, T8 reads 64-127; the
                        # 64-127 half of t1 holds A>>1, so offset-1 gives the
                        # same tap data)
                        for sp in window:
                            r0 = sp * ROWS
                            base = (r0 + 2) * PW + 2
                            for j in range(2):
                                wap = w22[:, (g * 2 + j) * 128 : (g * 2 + j + 1) * 128]
                                nc.tensor.matmul(
                                    ps[2 * j, sp][:],
                                    wap[0:64, :],
                                    t1[0:64, base : base + NCOL],
                                    start=False,
                                    stop=True,
                                    tile_position=(0, 0),
                                )
                                nc.tensor.matmul(
                                    ps[2 * j + 1, sp][:],
                                    wap[64:128, :],
                                    t1[64:128, base - 1 : base - 1 + NCOL],
                                    start=False,
                                    stop=True,
                                    tile_position=(64, 0),
                                )
                        for sp in window:
                            for oc4 in range(OC4):
                                iscl = g * OC4 + oc4
                                ratio_ap = scl[:, iscl : iscl + 1]
                                c_ap = scl[:, G * OC4 + iscl : G * OC4 + iscl + 1]
                                q8 = qp.tile([128, NCOL], I8, tag="q8")
                                nc.scalar.activation(
                                    q8[:],
                                    ps[oc4, sp][:],
                                    mybir.ActivationFunctionType.Copy,
                                    bias=0.0,
                                    scale=ratio_ap,
                                )
                                a = acc[oc4 % 2, sp]
                                if g == 0 and oc4 < 2:
                                    nc.vector.tensor_scalar(
                                        a[:], q8[:], c_ap, None, mybir.AluOpType.mult
                                    )
                                else:
                                    nc.vector.scalar_tensor_tensor(
                                        a[:],
                                        q8[:],
                                        c_ap,
                                        a[:],
                                        mybir.AluOpType.mult,
                                        mybir.AluOpType.add,
                                    )

                for oct in range(2):
                    for sp in range(SP):
                        a = acc[oct, sp]
                        eng = out_q[(oct * SP + sp) % 2]
                        eng.dma_start(out_d[img, oct, sp], a[:])

    nc.compile()
    return nc


def _prepare(x, weight, w_scale, ps_scale_p, ps_scale_n):
    x = np.asarray(x, np.float32)
    weight = np.asarray(weight, np.float32)
    w_scale = np.asarray(w_scale, np.float32)
    ps_scale_p = np.asarray(ps_scale_p, np.float32)
    ps_scale_n = np.asarray(ps_scale_n, np.float32)

    # --- weight levels (exact f32 math matching the reference LSQ) ---
    wg = weight.reshape(OC, G, CG, K, K).transpose(1, 0, 2, 3, 4)  # [G,O,cg,k,k]
    s_w = w_scale.reshape(G, 1, 1, 1, 1)
    lvl_p = np.round(np.clip(np.maximum(wg, 0) / s_w, 0.0, float(QP_W))).astype(np.float32)
    lvl_n = np.round(np.clip(np.maximum(-wg, 0) / s_w, 0.0, float(QP_W))).astype(np.float32)
    LV = np.concatenate([lvl_p, lvl_n], axis=1)  # [G, 512, cg, 3, 3]

    # lhsT main slots [K=128, M=128] per (g, oc4, slot 0..3)
    wts = np.zeros((G, OC4, 4, 128, 128), np.float16)
    # tap-(2,2) slots: per (g, pair j): even oc4 on partitions 0-63, odd on 64-127
    w22 = np.zeros((G, 2, 128, 128), np.float16)
    for g in range(G):
        for oc4 in range(OC4):
            t = LV[g, oc4 * 128 : (oc4 + 1) * 128]  # [128 och, cg, 3, 3]
            for s in range(3):  # taps (s,0)+(s,1)
                wts[g, oc4, s, :CG] = t[:, :, s, 0].T
                wts[g, oc4, s, CG:] = t[:, :, s, 1].T
            wts[g, oc4, 3, :CG] = t[:, :, 0, 2].T  # taps (0,2)+(1,2) via T2
            wts[g, oc4, 3, CG:] = t[:, :, 1, 2].T
            j, half = divmod(oc4, 2)
            w22[g, j, half * CG : half * CG + CG] = t[:, :, 2, 2].T
    wts_flat = np.ascontiguousarray(wts.transpose(3, 0, 1, 2, 4).reshape(128, G * OC4 * 4 * 128))
    w22_flat = np.ascontiguousarray(w22.transpose(2, 0, 1, 3).reshape(128, G * 2 * 128))

    # --- scales: ratio = s_w/s_ps ; c = +-s_ps ---
    scl = np.zeros((128, 2 * G * OC4), np.float32)
    for g in range(G):
        for oc4 in range(OC4):
            s_ps = ps_scale_p[g] if oc4 < 2 else ps_scale_n[g]
            sign = 1.0 if oc4 < 2 else -1.0
            scl[:, g * OC4 + oc4] = np.float32(w_scale[g]) / np.float32(s_ps)
            scl[:, G * OC4 + g * OC4 + oc4] = np.float32(sign) * np.float32(s_ps)

    # --- padded, shifted x in fp16 ---
    xp = np.zeros((B, IC, PH, PW), np.float16)
    xp[:, :, 1 : H + 1, 1 : W + 1] = x.astype(np.float16)
    Af = np.zeros((B, G, CG, FLATP), np.float16)
    Af[..., :FLAT] = xp.reshape(B, G, CG, FLAT)
    T1 = np.zeros((B, G, 128, FLATP), np.float16)
    T1[:, :, :CG] = Af
    T1[:, :, CG:, : FLATP - 1] = Af[..., 1:]
    T2 = np.zeros((B, G, 128, FLATP), np.float16)
    T2[:, :, :CG] = Af
    T2[:, :, CG:, : FLATP - PW] = Af[..., PW:]

    return T1, T2, wts_flat, w22_flat, scl


def kernel(x, weight, w_scale, ps_scale_p, ps_scale_n, _trace=False, _tmpdir=None):
    T1, T2, wts_flat, w22_flat, scl = _prepare(x, weight, w_scale, ps_scale_p, ps_scale_n)

    if "nc" not in _nc_cache:
        _nc_cache["nc"] = _build_nc()
    nc = _nc_cache["nc"]

    in_maps = []
    for c in range(N_CORES):
        sl = slice(c * BPC, (c + 1) * BPC)
        in_maps.append(
            {
                "xt1": np.ascontiguousarray(T1[sl]),
                "xt2": np.ascontiguousarray(T2[sl]),
                "wts": wts_flat,
                "w22": w22_flat,
                "scl": scl,
            }
        )

    kwargs = {}
    if _trace:
        kwargs.update(trace=True, tmpdir=_tmpdir, trace_cores=[0])
    res = run_bass_kernel_spmd(nc, in_maps, core_ids=list(range(N_CORES)), **kwargs)

    out = np.concatenate([r["out"] for r in res.results], axis=0)  # [16, 2, 7, 128, 464] fp16
    v = out.reshape(B, 2, SP, 128, ROWS, PW)[..., :W]  # strip pad cols
    final = np.ascontiguousarray(v.transpose(0, 1, 3, 2, 4, 5)).reshape(B, OC, H, W).astype(np.float32)
    if _trace:
        kernel._last_results = res
    return final


# revision 7
# speedup vs baseline: 1.0531x; 1.0531x over previous
"""Trainium2 Bass kernel for nn_Conv4Pim_group_arr_v3 (PIM-style grouped quantized conv).

Computation (see reference):
  - x [16,256,56,56] f32, weight [256,256,3,3], per-group (G=4, 64 ic each) LSQ
    quantization: weights to integer levels {0..3} (pos/neg split), partial-sum conv
    outputs rounded to int levels in [-128,127] and rescaled, accumulated over groups.

Strategy: data-parallel over batch (2 images per core, 8 cores, no collectives).
Per core, per (img, group, och-tile-of-512, sptile-of-8-rows):
  - 4 full fp16 matmuls (K=128: two conv taps x 64 ic stacked; x stored as [A | A+1]
    and [A | A+58] shifted copies so taps pair up) accumulate 8 of the 9 conv taps
    into one PSUM tile [128 och, 464=8*58 padded-row columns].
  - The 9th tap (2,2) is K=64 only; two och-tiles' tap-(2,2) matmuls are packed into
    one PE pass via 64x128 row tiling (tile_position (0,0)/(64,0)), batched per
    2-sp window to amortize the PE mode-switch drain.
  - ACT: Copy(psum * (w_scale/ps_scale)) with int8 output = round-half-even +
    saturate to [-128,127] in one op (verified on HW) == the LSQ psum quantizer.
  - DVE scalar_tensor_tensor: acc_fp16 += q_int8 * (+-ps_scale).
Output fp16 -> host f32.
"""

import numpy as np

import concourse.mybir as mybir
import concourse.tile as tile
from concourse import bacc
from concourse.bass_utils import run_bass_kernel_spmd

F32 = mybir.dt.float32
F16 = mybir.dt.float16
I8 = mybir.dt.int8

B, IC, H, W = 16, 256, 56, 56
OC = 256
G = 4
CG = 64  # ic per group
K = 3
QP_W = 3  # 2**2 - 1
N_CORES = 8
BPC = B // N_CORES  # images per core

PW = W + 2  # 58 padded width
PH = H + 2
FLAT = PW * PH  # 3364
FLATP = FLAT + 4  # padded to 3368 for tap-read overhang
SP = 7  # spatial tiles of 8 output rows
ROWS = 8
NCOL = ROWS * PW  # 464 columns per psum tile
OC4 = 4  # och tiles of 128 over 512 (pos|neg x 256)
WINDOWS = [(0, 1), (2, 3), (4, 5), (6,)]

_nc_cache = {}


def _build_nc():
    nc = bacc.Bacc(
        "TRN2",
        target_bir_lowering=False,
        debug=False,
        enable_asserts=True,
        num_devices=N_CORES,
    )

    xt1_d = nc.dram_tensor("xt1", [BPC, G, 128, FLATP], F16, kind="ExternalInput").ap()
    xt2_d = nc.dram_tensor("xt2", [BPC, G, 128, FLATP], F16, kind="ExternalInput").ap()
    # main slots: 4 per (g, oc4); tap-(2,2) packed separately for row tiling
    wts_d = nc.dram_tensor("wts", [128, G * OC4 * 4 * 128], F16, kind="ExternalInput").ap()
    w22_d = nc.dram_tensor("w22", [128, G * 2 * 128], F16, kind="ExternalInput").ap()
    scl_d = nc.dram_tensor("scl", [128, 2 * G * OC4], F32, kind="ExternalInput").ap()
    # padded output: [img, oct, sp, och, 464 cols]; host strips the pad columns
    out_d = nc.dram_tensor("out", [BPC, 2, SP, 128, NCOL], F16, kind="ExternalOutput").ap()

    W1 = 4 * 128  # one (g, oc4) main-slot weight slice
    WG = OC4 * W1  # one group of main slots

    with tile.TileContext(nc) as tc:
        with (
            tc.tile_pool(name="xp", bufs=1) as xp,
            tc.tile_pool(name="wp", bufs=1) as wp,
            tc.tile_pool(name="accp", bufs=2) as accp,
            tc.tile_pool(name="qp", bufs=8) as qp,
            tc.tile_pool(name="psum", bufs=8, space="PSUM") as pp,
        ):
            wts = wp.tile([128, G * OC4 * 4 * 128], F16, tag="wts")
            w22 = wp.tile([128, G * 2 * 128], F16, tag="w22")
            scl = wp.tile([128, 2 * G * OC4], F32, tag="scl")

            # Startup-critical DMA schedule over four queues (sync + scalar =
            # HWDGE, gpsimd q0/q1 = SWDGE). First window (img0,g0,sp0-1) needs
            # t1/t2 cols < 1046 and the g0 weight slices; chunks ordered to
            # stay ahead of consumption.
            C1, C2, C3 = 582, 1046, 2500

            xt = {}
            t1_first = xp.tile([128, FLATP], F16, tag="t1_0_0")
            t2_first = xp.tile([128, FLATP], F16, tag="t2_0_0")
            xt[0, 0] = (t1_first, t2_first)

            nc.sync.dma_start(scl[:], scl_d[:])
            nc.sync.dma_start(wts[:, :W1], wts_d[:, :W1])
            nc.sync.dma_start(t1_first[:, :C1], xt1_d[0, 0, :, :C1])
            for i in range(1, OC4):
                nc.sync.dma_start(wts[:, i * W1 : (i + 1) * W1], wts_d[:, i * W1 : (i + 1) * W1])
            nc.sync.dma_start(t1_first[:, C1:C2], xt1_d[0, 0, :, C1:C2])
            nc.sync.dma_start(t1_first[:, C2:C3], xt1_d[0, 0, :, C2:C3])
            nc.sync.dma_start(t1_first[:, C3:], xt1_d[0, 0, :, C3:])

            # scalar HWDGE: w22 (needed by the window-opening tap22 MMs) + the
            # first t2 chunks; issued before the scalar engine's first ACT op,
            # transfers run async. Nothing queued here later (keeps ACT clean).
            nc.scalar.dma_start(w22[:, :256], w22_d[:, :256])
            nc.scalar.dma_start(t2_first[:, :C1], xt2_d[0, 0, :, :C1])
            nc.scalar.dma_start(t2_first[:, C1:C2], xt2_d[0, 0, :, C1:C2])
            nc.scalar.dma_start(w22[:, 256:], w22_d[:, 256:])

            # gpsimd q0: rest of first-set t2, then per-set t2 streams with
            # weight groups just-in-time
            nc.gpsimd.dma_start(t2_first[:, C2:C3], xt2_d[0, 0, :, C2:C3])
            nc.gpsimd.dma_start(t2_first[:, C3:], xt2_d[0, 0, :, C3:])
            nc.gpsimd.dma_start(wts[:, WG : 2 * WG], wts_d[:, WG : 2 * WG])

            for img in range(BPC):
                for g in range(G):
                    if (img, g) in xt:
                        continue
                    t1 = xp.tile([128, FLATP], F16, tag=f"t1_{img}_{g}")
                    t2 = xp.tile([128, FLATP], F16, tag=f"t2_{img}_{g}")
                    nc.sync.dma_start(t1[:], xt1_d[img, g])
                    nc.gpsimd.dma_start(t2[:], xt2_d[img, g])
                    xt[img, g] = (t1, t2)
                    if (img, g) == (0, 1):
                        # remaining weights after the (0,1) x tiles
                        nc.gpsimd.dma_start(wts[:, 2 * WG :], wts_d[:, 2 * WG :])

            def wslice(g, oc4, s):
                i = ((g * OC4) + oc4) * 4 + s
                return wts[:, i * 128 : (i + 1) * 128]

            out_q = [nc.sync, nc.gpsimd]

            for img in range(BPC):
                acc = {}
                for oct in range(2):
                    for sp in range(SP):
                        a_t = accp.tile([128, NCOL], F16, tag=f"acc{oct}_{sp}")
                        acc[oct, sp] = a_t

                for g in range(G):
                    t1, t2 = xt[img, g]
                    for window in WINDOWS:
                        ps = {}
                        # tap-(2,2) first: opens each PSUM bank (start=True) via
                        # 64x128 row-tiled pairs (T0 reads partitions 0-63, T8
                        # reads 64-127; the 64-127 half of t1 holds A>>1, so
                        # offset-1 gives the same tap data). Banks then close
                        # progressively during the main-slot MMs so the ACT
                        # quantize runs at a steady cadence, not in bursts.
                        for sp in window:
                            for oc4 in range(OC4):
                                p = pp.tile([128, NCOL], F32, tag="ps", name="ps")
                                ps[oc4, sp] = p
                        for sp in window:
                            r0 = sp * ROWS
                            base = (r0 + 2) * PW + 2
                            for j in range(2):
                                wap = w22[:, (g * 2 + j) * 128 : (g * 2 + j + 1) * 128]
                                nc.tensor.matmul(
                                    ps[2 * j, sp][:],
                                    wap[0:64, :],
                                    t1[0:64, base : base + NCOL],
                                    start=True,
                                    stop=False,
                                    tile_position=(0, 0),
                                )
                                nc.tensor.matmul(
                                    ps[2 * j + 1, sp][:],
                                    wap[64:128, :],
                                    t1[64:128, base - 1 : base - 1 + NCOL],
                                    start=True,
                                    stop=False,
                                    tile_position=(64, 0),
                                )
                        for sp in window:
                            r0 = sp * ROWS
                            for oc4 in range(OC4):
                                p = ps[oc4, sp]
                                for s in range(3):
                                    nc.tensor.matmul(
                                        p[:],
                                        wslice(g, oc4, s),
                                        t1[:, (r0 + s) * PW : (r0 + s) * PW + NCOL],
                                        start=False,
                                        stop=False,
                                    )
                                nc.tensor.matmul(
                                    p[:],
                                    wslice(g, oc4, 3),
                                    t2[:, r0 * PW + 2 : r0 * PW + 2 + NCOL],
                                    start=False,
                                    stop=True,
                                )
                                iscl = g * OC4 + oc4
                                ratio_ap = scl[:, iscl : iscl + 1]
                                c_ap = scl[:, G * OC4 + iscl : G * OC4 + iscl + 1]
                                q8 = qp.tile([128, NCOL], I8, tag="q8")
                                nc.scalar.activation(
                                    q8[:],
                                    p[:],
                                    mybir.ActivationFunctionType.Copy,
                                    bias=0.0,
                                    scale=ratio_ap,
                                )
                                a = acc[oc4 % 2, sp]
                                if g == 0 and oc4 < 2:
                                    nc.vector.tensor_scalar(
                                        a[:], q8[:], c_ap, None, mybir.AluOpType.mult
                                    )
                                else:
                                    nc.vector.scalar_tensor_tensor(
                                        a[:],
                                        q8[:],
                                        c_ap,
                                        a[:],
                                        mybir.AluOpType.mult,
                                        mybir.AluOpType.add,
                                    )

                for oct in range(2):
                    for sp in range(SP):
                        a = acc[oct, sp]
                        eng = out_q[(oct * SP + sp) % 2]
                        eng.dma_start(out_d[img, oct, sp], a[:])

    nc.compile()
    return nc


def _prepare(x, weight, w_scale, ps_scale_p, ps_scale_n):
    x = np.asarray(x, np.float32)
    weight = np.asarray(weight, np.float32)
    w_scale = np.asarray(w_scale, np.float32)
    ps_scale_p = np.asarray(ps_scale_p, np.float32)
    ps_scale_n = np.asarray(ps_scale_n, np.float32)

    # --- weight levels (exact f32 math matching the reference LSQ) ---
    wg = weight.reshape(OC, G, CG, K, K).transpose(1, 0, 2, 3, 4)  # [G,O,cg,k,k]
    s_w = w_scale.reshape(G, 1, 1, 1, 1)
    lvl_p = np.round(np.clip(np.maximum(wg, 0) / s_w, 0.0, float(QP_W))).astype(np.float32)
    lvl_n = np.round(np.clip(np.maximum(-wg, 0) / s_w, 0.0, float(QP_W))).astype(np.float32)
    LV = np.concatenate([lvl_p, lvl_n], axis=1)  # [G, 512, cg, 3, 3]

    # lhsT main slots [K=128, M=128] per (g, oc4, slot 0..3)
    wts = np.zeros((G, OC4, 4, 128, 128), np.float16)
    # tap-(2,2) slots: per (g, pair j): even oc4 on partitions 0-63, odd on 64-127
    w22 = np.zeros((G, 2, 128, 128), np.float16)
    for g in range(G):
        for oc4 in range(OC4):
            t = LV[g, oc4 * 128 : (oc4 + 1) * 128]  # [128 och, cg, 3, 3]
            for s in range(3):  # taps (s,0)+(s,1)
                wts[g, oc4, s, :CG] = t[:, :, s, 0].T
                wts[g, oc4, s, CG:] = t[:, :, s, 1].T
            wts[g, oc4, 3, :CG] = t[:, :, 0, 2].T  # taps (0,2)+(1,2) via T2
            wts[g, oc4, 3, CG:] = t[:, :, 1, 2].T
            j, half = divmod(oc4, 2)
            w22[g, j, half * CG : half * CG + CG] = t[:, :, 2, 2].T
    wts_flat = np.ascontiguousarray(wts.transpose(3, 0, 1, 2, 4).reshape(128, G * OC4 * 4 * 128))
    w22_flat = np.ascontiguousarray(w22.transpose(2, 0, 1, 3).reshape(128, G * 2 * 128))

    # --- scales: ratio = s_w/s_ps ; c = +-s_ps ---
    scl = np.zeros((128, 2 * G * OC4), np.float32)
    for g in range(G):
        for oc4 in range(OC4):
            s_ps = ps_scale_p[g] if oc4 < 2 else ps_scale_n[g]
            sign = 1.0 if oc4 < 2 else -1.0
            scl[:, g * OC4 + oc4] = np.float32(w_scale[g]) / np.float32(s_ps)
            scl[:, G * OC4 + g * OC4 + oc4] = np.float32(sign) * np.float32(s_ps)

    # --- padded, shifted x in fp16 ---
    xp = np.zeros((B, IC, PH, PW), np.float16)
    xp[:, :, 1 : H + 1, 1 : W + 1] = x.astype(np.float16)
    Af = np.zeros((B, G, CG, FLATP), np.float16)
    Af[..., :FLAT] = xp.reshape(B, G, CG, FLAT)
    T1 = np.zeros((B, G, 128, FLATP), np.float16)
    T1[:, :, :CG] = Af
    T1[:, :, CG:, : FLATP - 1] = Af[..., 1:]
    T2 = np.zeros((B, G, 128, FLATP), np.float16)
    T2[:, :, :CG] = Af
    T2[:, :, CG:, : FLATP - PW] = Af[..., PW:]

    return T1, T2, wts_flat, w22_flat, scl


def kernel(x, weight, w_scale, ps_scale_p, ps_scale_n, _trace=False, _tmpdir=None):
    T1, T2, wts_flat, w22_flat, scl = _prepare(x, weight, w_scale, ps_scale_p, ps_scale_n)

    if "nc" not in _nc_cache:
        _nc_cache["nc"] = _build_nc()
    nc = _nc_cache["nc"]

    in_maps = []
    for c in range(N_CORES):
        sl = slice(c * BPC, (c + 1) * BPC)
        in_maps.append(
            {
                "xt1": np.ascontiguousarray(T1[sl]),
                "xt2": np.ascontiguousarray(T2[sl]),
                "wts": wts_flat,
                "w22": w22_flat,
                "scl": scl,
            }
        )

    kwargs = {}
    if _trace:
        kwargs.update(trace=True, tmpdir=_tmpdir, trace_cores=[0])
    res = run_bass_kernel_spmd(nc, in_maps, core_ids=list(range(N_CORES)), **kwargs)

    out = np.concatenate([r["out"] for r in res.results], axis=0)  # [16, 2, 7, 128, 464] fp16
    v = out.reshape(B, 2, SP, 128, ROWS, PW)[..., :W]  # strip pad cols
    final = np.ascontiguousarray(v.transpose(0, 1, 3, 2, 4, 5)).reshape(B, OC, H, W).astype(np.float32)
    if _trace:
        kernel._last_results = res
    return final


# revision 8
# speedup vs baseline: 1.0863x; 1.0314x over previous
"""Trainium2 Bass kernel for nn_Conv4Pim_group_arr_v3 (PIM-style grouped quantized conv).

Computation (see reference):
  - x [16,256,56,56] f32, weight [256,256,3,3], per-group (G=4, 64 ic each) LSQ
    quantization: weights to integer levels {0..3} (pos/neg split), partial-sum conv
    outputs rounded to int levels in [-128,127] and rescaled, accumulated over groups.

Strategy: data-parallel over batch (2 images per core, 8 cores, no collectives).
Per core, per (img, group, och-tile-of-512, sptile-of-8-rows):
  - 4 full fp16 matmuls (K=128: two conv taps x 64 ic stacked; x stored as [A | A+1]
    and [A | A+58] shifted copies so taps pair up) accumulate 8 of the 9 conv taps
    into one PSUM tile [128 och, 464=8*58 padded-row columns].
  - The 9th tap (2,2) is K=64 only; two och-tiles' tap-(2,2) matmuls are packed into
    one PE pass via 64x128 row tiling (tile_position (0,0)/(64,0)), batched per
    2-sp window to amortize the PE mode-switch drain.
  - ACT: Copy(psum * (w_scale/ps_scale)) with int8 output = round-half-even +
    saturate to [-128,127] in one op (verified on HW) == the LSQ psum quantizer.
  - DVE scalar_tensor_tensor: acc_fp16 += q_int8 * (+-ps_scale).
Output fp16 -> host f32.
"""

import numpy as np

import concourse.mybir as mybir
import concourse.tile as tile
from concourse import bacc
from concourse.bass_utils import run_bass_kernel_spmd

F32 = mybir.dt.float32
F16 = mybir.dt.float16
I8 = mybir.dt.int8

B, IC, H, W = 16, 256, 56, 56
OC = 256
G = 4
CG = 64  # ic per group
K = 3
QP_W = 3  # 2**2 - 1
N_CORES = 8
BPC = B // N_CORES  # images per core

PW = W + 2  # 58 padded width
PH = H + 2
FLAT = PW * PH  # 3364
FLATP = FLAT + 4  # padded to 3368 for tap-read overhang
SP = 7  # spatial tiles of 8 output rows
ROWS = 8
NCOL = ROWS * PW  # 464 columns per psum tile
OC4 = 4  # och tiles of 128 over 512 (pos|neg x 256)
WINDOWS = [(0, 1), (2, 3), (4, 5), (6,)]

_nc_cache = {}


def _build_nc():
    nc = bacc.Bacc(
        "TRN2",
        target_bir_lowering=False,
        debug=False,
        enable_asserts=True,
        num_devices=N_CORES,
    )

    xt1_d = nc.dram_tensor("xt1", [BPC, G, 128, FLATP], F16, kind="ExternalInput").ap()
    xt2_d = nc.dram_tensor("xt2", [BPC, G, 128, FLATP], F16, kind="ExternalInput").ap()
    # main slots: 4 per (g, oc4); tap-(2,2) packed separately for row tiling
    wts_d = nc.dram_tensor("wts", [128, G * OC4 * 4 * 128], F16, kind="ExternalInput").ap()
    w22_d = nc.dram_tensor("w22", [128, G * 2 * 128], F16, kind="ExternalInput").ap()
    scl_d = nc.dram_tensor("scl", [128, 2 * G * OC4], F32, kind="ExternalInput").ap()
    # padded output: [img, oct, sp, och, 464 cols]; host strips the pad columns
    out_d = nc.dram_tensor("out", [BPC, 2, SP, 128, NCOL], F16, kind="ExternalOutput").ap()

    W1 = 4 * 128  # one (g, oc4) main-slot weight slice
    WG = OC4 * W1  # one group of main slots

    with tile.TileContext(nc) as tc:
        with (
            tc.tile_pool(name="xp", bufs=1) as xp,
            tc.tile_pool(name="wp", bufs=1) as wp,
            tc.tile_pool(name="accp", bufs=2) as accp,
            tc.tile_pool(name="qp", bufs=8) as qp,
            tc.tile_pool(name="psum", bufs=8, space="PSUM") as pp,
        ):
            wts = wp.tile([128, G * OC4 * 4 * 128], F16, tag="wts")
            w22 = wp.tile([128, G * 2 * 128], F16, tag="w22")
            scl = wp.tile([128, 2 * G * OC4], F32, tag="scl")

            # Startup-critical DMA schedule over four queues (sync + scalar =
            # HWDGE, gpsimd q0/q1 = SWDGE). First window (img0,g0,sp0-1) needs
            # t1/t2 cols < 1046 and the g0 weight slices; chunks ordered to
            # stay ahead of consumption.
            C1, C2, C3 = 582, 1046, 2500

            xt = {}
            t1_first = xp.tile([128, FLATP], F16, tag="t1_0_0")
            t2_first = xp.tile([128, FLATP], F16, tag="t2_0_0")
            xt[0, 0] = (t1_first, t2_first)

            # DMA issues cost ~700ns of engine time each and serialize per
            # engine, so the first issue on each of the three queues is what
            # the PE needs first: tap22 needs w22 (scalar#1) + t1c1 (sync#1);
            # the main slots then need wts slices (sync#2..5) and t2c1
            # (gpsimd#1), paced one window ahead.
            nc.sync.dma_start(t1_first[:, :C1], xt1_d[0, 0, :, :C1])
            nc.sync.dma_start(wts[:, :W1], wts_d[:, :W1])
            for i in range(1, OC4):
                nc.sync.dma_start(wts[:, i * W1 : (i + 1) * W1], wts_d[:, i * W1 : (i + 1) * W1])
            nc.sync.dma_start(t1_first[:, C2:C3], xt1_d[0, 0, :, C2:C3])
            nc.sync.dma_start(t1_first[:, C3:], xt1_d[0, 0, :, C3:])

            # scalar HWDGE: w22 (window-opening tap22 MMs), ACT scales, second
            # t1 chunk; all issued before the scalar engine's first ACT op.
            nc.scalar.dma_start(w22[:, :256], w22_d[:, :256])
            nc.scalar.dma_start(scl[:], scl_d[:])
            nc.scalar.dma_start(t1_first[:, C1:C2], xt1_d[0, 0, :, C1:C2])
            nc.scalar.dma_start(w22[:, 256:], w22_d[:, 256:])

            # gpsimd q0: first-set t2 stream, then weight groups just-in-time
            nc.gpsimd.dma_start(t2_first[:, :C1], xt2_d[0, 0, :, :C1])
            nc.gpsimd.dma_start(t2_first[:, C1:C2], xt2_d[0, 0, :, C1:C2])
            nc.gpsimd.dma_start(t2_first[:, C2:C3], xt2_d[0, 0, :, C2:C3])
            nc.gpsimd.dma_start(t2_first[:, C3:], xt2_d[0, 0, :, C3:])
            nc.gpsimd.dma_start(wts[:, WG : 2 * WG], wts_d[:, WG : 2 * WG])

            for img in range(BPC):
                for g in range(G):
                    if (img, g) in xt:
                        continue
                    t1 = xp.tile([128, FLATP], F16, tag=f"t1_{img}_{g}")
                    t2 = xp.tile([128, FLATP], F16, tag=f"t2_{img}_{g}")
                    nc.sync.dma_start(t1[:], xt1_d[img, g])
                    nc.gpsimd.dma_start(t2[:], xt2_d[img, g])
                    xt[img, g] = (t1, t2)
                    if (img, g) == (0, 1):
                        # remaining weights after the (0,1) x tiles
                        nc.gpsimd.dma_start(wts[:, 2 * WG :], wts_d[:, 2 * WG :])

            def wslice(g, oc4, s):
                i = ((g * OC4) + oc4) * 4 + s
                return wts[:, i * 128 : (i + 1) * 128]

            out_q = [nc.sync, nc.gpsimd]

            for img in range(BPC):
                acc = {}
                for oct in range(2):
                    for sp in range(SP):
                        a_t = accp.tile([128, NCOL], F16, tag=f"acc{oct}_{sp}")
                        acc[oct, sp] = a_t

                for g in range(G):
                    t1, t2 = xt[img, g]
                    for window in WINDOWS:
                        ps = {}
                        # tap-(2,2) first: opens each PSUM bank (start=True) via
                        # 64x128 row-tiled pairs (T0 reads partitions 0-63, T8
                        # reads 64-127; the 64-127 half of t1 holds A>>1, so
                        # offset-1 gives the same tap data). Banks then close
                        # progressively during the main-slot MMs so the ACT
                        # quantize runs at a steady cadence, not in bursts.
                        for sp in window:
                            for oc4 in range(OC4):
                                p = pp.tile([128, NCOL], F32, tag="ps", name="ps")
                                ps[oc4, sp] = p
                        for sp in window:
                            r0 = sp * ROWS
                            base = (r0 + 2) * PW + 2
                            for j in range(2):
                                wap = w22[:, (g * 2 + j) * 128 : (g * 2 + j + 1) * 128]
                                nc.tensor.matmul(
                                    ps[2 * j, sp][:],
                                    wap[0:64, :],
                                    t1[0:64, base : base + NCOL],
                                    start=True,
                                    stop=False,
                                    tile_position=(0, 0),
                                )
                                nc.tensor.matmul(
                                    ps[2 * j + 1, sp][:],
                                    wap[64:128, :],
                                    t1[64:128, base - 1 : base - 1 + NCOL],
                                    start=True,
                                    stop=False,
                                    tile_position=(64, 0),
                                )
                        for sp in window:
                            r0 = sp * ROWS
                            for oc4 in range(OC4):
                                p = ps[oc4, sp]
                                for s in range(3):
                                    nc.tensor.matmul(
                                        p[:],
                                        wslice(g, oc4, s),
                                        t1[:, (r0 + s) * PW : (r0 + s) * PW + NCOL],
                                        start=False,
                                        stop=False,
                                    )
                                nc.tensor.matmul(
                                    p[:],
                                    wslice(g, oc4, 3),
                                    t2[:, r0 * PW + 2 : r0 * PW + 2 + NCOL],
                                    start=False,
                                    stop=True,
                                )
                                iscl = g * OC4 + oc4
                                ratio_ap = scl[:, iscl : iscl + 1]
                                c_ap = scl[:, G * OC4 + iscl : G * OC4 + iscl + 1]
                                q8 = qp.tile([128, NCOL], I8, tag="q8")
                                nc.scalar.activation(
                                    q8[:],
                                    p[:],
                                    mybir.ActivationFunctionType.Copy,
                                    bias=0.0,
                                    scale=ratio_ap,
                                )
                                a = acc[oc4 % 2, sp]
                                if g == 0 and oc4 < 2:
                                    nc.vector.tensor_scalar(
                                        a[:], q8[:], c_ap, None, mybir.AluOpType.mult
                                    )
                                else:
                                    nc.vector.scalar_tensor_tensor(
                                        a[:],
                                        q8[:],
                                        c_ap,
                                        a[:],
                                        mybir.AluOpType.mult,
                                        mybir.AluOpType.add,
                                    )

                for oct in range(2):
                    for sp in range(SP):
                        a = acc[oct, sp]
                        eng = out_q[(oct * SP + sp) % 2]
                        eng.dma_start(out_d[img, oct, sp], a[:])

    nc.compile()
    return nc


def _prepare(x, weight, w_scale, ps_scale_p, ps_scale_n):
    x = np.asarray(x, np.float32)
    weight = np.asarray(weight, np.float32)
    w_scale = np.asarray(w_scale, np.float32)
    ps_scale_p = np.asarray(ps_scale_p, np.float32)
    ps_scale_n = np.asarray(ps_scale_n, np.float32)

    # --- weight levels (exact f32 math matching the reference LSQ) ---
    wg = weight.reshape(OC, G, CG, K, K).transpose(1, 0, 2, 3, 4)  # [G,O,cg,k,k]
    s_w = w_scale.reshape(G, 1, 1, 1, 1)
    lvl_p = np.round(np.clip(np.maximum(wg, 0) / s_w, 0.0, float(QP_W))).astype(np.float32)
    lvl_n = np.round(np.clip(np.maximum(-wg, 0) / s_w, 0.0, float(QP_W))).astype(np.float32)
    LV = np.concatenate([lvl_p, lvl_n], axis=1)  # [G, 512, cg, 3, 3]

    # lhsT main slots [K=128, M=128] per (g, oc4, slot 0..3)
    wts = np.zeros((G, OC4, 4, 128, 128), np.float16)
    # tap-(2,2) slots: per (g, pair j): even oc4 on partitions 0-63, odd on 64-127
    w22 = np.zeros((G, 2, 128, 128), np.float16)
    for g in range(G):
        for oc4 in range(OC4):
            t = LV[g, oc4 * 128 : (oc4 + 1) * 128]  # [128 och, cg, 3, 3]
            for s in range(3):  # taps (s,0)+(s,1)
                wts[g, oc4, s, :CG] = t[:, :, s, 0].T
                wts[g, oc4, s, CG:] = t[:, :, s, 1].T
            wts[g, oc4, 3, :CG] = t[:, :, 0, 2].T  # taps (0,2)+(1,2) via T2
            wts[g, oc4, 3, CG:] = t[:, :, 1, 2].T
            j, half = divmod(oc4, 2)
            w22[g, j, half * CG : half * CG + CG] = t[:, :, 2, 2].T
    wts_flat = np.ascontiguousarray(wts.transpose(3, 0, 1, 2, 4).reshape(128, G * OC4 * 4 * 128))
    w22_flat = np.ascontiguousarray(w22.transpose(2, 0, 1, 3).reshape(128, G * 2 * 128))

    # --- scales: ratio = s_w/s_ps ; c = +-s_ps ---
    scl = np.zeros((128, 2 * G * OC4), np.float32)
    for g in range(G):
        for oc4 in range(OC4):
            s_ps = ps_scale_p[g] if oc4 < 2 else ps_scale_n[g]
            sign = 1.0 if oc4 < 2 else -1.0
            scl[:, g * OC4 + oc4] = np.float32(w_scale[g]) / np.float32(s_ps)
            scl[:, G * OC4 + g * OC4 + oc4] = np.float32(sign) * np.float32(s_ps)

    # --- padded, shifted x in fp16 ---
    xp = np.zeros((B, IC, PH, PW), np.float16)
    xp[:, :, 1 : H + 1, 1 : W + 1] = x.astype(np.float16)
    Af = np.zeros((B, G, CG, FLATP), np.float16)
    Af[..., :FLAT] = xp.reshape(B, G, CG, FLAT)
    T1 = np.zeros((B, G, 128, FLATP), np.float16)
    T1[:, :, :CG] = Af
    T1[:, :, CG:, : FLATP - 1] = Af[..., 1:]
    T2 = np.zeros((B, G, 128, FLATP), np.float16)
    T2[:, :, :CG] = Af
    T2[:, :, CG:, : FLATP - PW] = Af[..., PW:]

    return T1, T2, wts_flat, w22_flat, scl


def kernel(x, weight, w_scale, ps_scale_p, ps_scale_n, _trace=False, _tmpdir=None):
    T1, T2, wts_flat, w22_flat, scl = _prepare(x, weight, w_scale, ps_scale_p, ps_scale_n)

    if "nc" not in _nc_cache:
        _nc_cache["nc"] = _build_nc()
    nc = _nc_cache["nc"]

    in_maps = []
    for c in range(N_CORES):
        sl = slice(c * BPC, (c + 1) * BPC)
        in_maps.append(
            {
                "xt1": np.ascontiguousarray(T1[sl]),
                "xt2": np.ascontiguousarray(T2[sl]),
                "wts": wts_flat,
                "w22": w22_flat,
                "scl": scl,
            }
        )

    kwargs = {}
    if _trace:
        kwargs.update(trace=True, tmpdir=_tmpdir, trace_cores=[0])
    res = run_bass_kernel_spmd(nc, in_maps, core_ids=list(range(N_CORES)), **kwargs)

    out = np.concatenate([r["out"] for r in res.results], axis=0)  # [16, 2, 7, 128, 464] fp16
    v = out.reshape(B, 2, SP, 128, ROWS, PW)[..., :W]  # strip pad cols
    final = np.ascontiguousarray(v.transpose(0, 1, 3, 2, 4, 5)).reshape(B, OC, H, W).astype(np.float32)
    if _trace:
        kernel._last_results = res
    return final
